# revision 4
# baseline (speedup 1.0000x reference)
"""Trainium2 Bass kernel for nn_CrossAttention_43061342110469.

Mathematical reduction: the reference's second einsum
    attn = einsum('bvhd,bhqk->bvhd', v, scores)
shares no contraction index with v, so it multiplies v elementwise by
S[b,h] = sum_{q,k} scores[b,h,q,k].  scores is a softmax over k, so every
row sums to 1 and S[b,h] == L == 2048 (verified: the fp32 reference
computes S == 2048.0 bit-exactly; end-to-end rel-err of the reduction is
~5e-7).  Therefore:

    out = (x @ Wv + bv) * 2048 @ Wo + bo
        = x @ (2048 * Wv @ Wo) + (2048 * bv @ Wo + bo)
        = x @ W' + b'

W' (1024x1024) and b' are folded on the host (float64 GEMM, ~ms), so the
device runs a single 8192x1024x1024 GEMM, row-sharded: 1024 rows per
core.  x and W' are cast to bf16 on the host (rel-err of the bf16 path vs
the fp32 reference is ~2.4e-3, tolerance is 2e-2); x is also pre-transposed
per-shard on the host so no on-device transposes are needed (matmul wants
the contraction dim on partitions for both operands).

Per-core device program:
  - DMA in: xT [1024d, 1024r] bf16 and W' [1024d, 1024n] bf16, chunked by
    128-row contraction tiles (256 KB each), pairs in ko order round-robin
    over 3 issue queues with depth-2 completion chains; bias b' broadcast.
  - ~3.4us of dummy matmuls from t0 so the PE HAM clock-gate opens while
    the first chunks land.
  - Phase A (rows 0-511): ko-outer accumulation over all 8 PSUM banks
    (4 row-tiles x 2 512-col halves) so contraction step ko runs as soon
    as chunk pair ko lands.
  - Phase B (rows 512-1023): all data resident; K-contiguous per row-tile
    so output tiles complete early and stream out.
  - Copies PSUM->SBUF fuse the bias add (DVE tensor_tensor); out tiles DMA
    to HBM round-robin over the 3 queues.
q/k/softmax are numerically dead and not computed.
"""

import sys

import numpy as np

_REPO = "/opt/trn_rl_repo"
if _REPO not in sys.path:
    sys.path.insert(0, _REPO)

B, L, D = 4, 2048, 1024
NCORES = 8
ROWS = B * L  # 8192
R = ROWS // NCORES  # 1024 rows per core
P = 128
NT = 512  # matmul free-dim tile (one PSUM bank of fp32)
KO = D // P  # 8 contraction tiles
RT = R // P  # 8 row tiles

_NC_CACHE = {}


def build_nc():
    """Build + compile the per-core Bass program (cached)."""
    if "nc" in _NC_CACHE:
        return _NC_CACHE["nc"]

    from contextlib import ExitStack

    import concourse.tile as tile
    from concourse import bacc, mybir
    from concourse.tile_rust import add_dep_helper
    from concourse._compat import get_trn_type

    f32 = mybir.dt.float32
    bf16 = mybir.dt.bfloat16

    nc = bacc.Bacc(
        get_trn_type() or "TRN2",
        target_bir_lowering=False,
        debug=False,
        num_devices=NCORES,
    )

    xt_nd = nc.dram_tensor("xt", [D, R], bf16, kind="ExternalInput").ap()
    w_nd = nc.dram_tensor("w", [D, D], bf16, kind="ExternalInput").ap()
    b_nd = nc.dram_tensor("b", [D], f32, kind="ExternalInput").ap()
    out_nd = nc.dram_tensor("out", [R, D], f32, kind="ExternalOutput").ap()

    with tile.TileContext(nc) as tc, ExitStack() as ctx:
        const = ctx.enter_context(tc.tile_pool(name="const", bufs=1))
        big = ctx.enter_context(tc.tile_pool(name="big", bufs=1))
        psp = ctx.enter_context(tc.tile_pool(name="psp", bufs=8, space="PSUM"))
        outp = ctx.enter_context(tc.tile_pool(name="outp", bufs=4))

        xt_sb = big.tile([P, KO, R], bf16)  # (2048*x)^T, [d_in, d_out, row]
        w_sb = big.tile([P, KO, D], bf16)  # W' as [d_in, d_out, n]
        b_rep = const.tile([P, D], f32)

        # --- DMA schedule: (xT chunk ko, W chunk ko) pairs in ko order,
        # round-robin over the 3 issue queues, depth-2 completion chains
        # (without them every dma_start floods the 16-engine fabric at once
        # and early chunks land late).
        xt_r = xt_nd.rearrange("(ko p) r -> p ko r", p=P)
        w_r = w_nd.rearrange("(ko p) n -> p ko n", p=P)
        jobs = []
        for ko in range(KO):
            jobs.append((xt_sb[:, ko], xt_r[:, ko]))
            jobs.append((w_sb[:, ko], w_r[:, ko]))

        qs = [nc.sync, nc.scalar, nc.gpsimd]
        chains = [[], [], []]

        def chained_dma(qi, dst, srcap):
            inst = qs[qi].dma_start(dst, srcap)
            ch = chains[qi]
            if len(ch) == 1:
                add_dep_helper(inst.ins, ch[-1].ins, sync=True, reason="dma chain")
            elif len(ch) >= 2:
                add_dep_helper(inst.ins, ch[-2].ins, sync=True, reason="dma chain")
            ch.append(inst)
            return inst

        for i, (dst, srcap) in enumerate(jobs):
            chained_dma(i % 3, dst, srcap)
        chained_dma(1, b_rep[:], b_nd[None, :].to_broadcast((P, D)))

        # --- PE warmup: ~3.5us of dummy matmuls from t0 so the HAM
        # clock-gate opens (K=8/8 @ 2.4GHz) by the time real work arrives.
        warm = const.tile([P, P], bf16)
        nc.vector.memset(warm[:], 1.0)
        wps = psp.tile([P, NT], f32, tag="t", name="wps")
        for _ in range(32):
            nc.tensor.matmul(
                wps[:, 0:P], lhsT=warm[:], rhs=warm[:], start=True, stop=True
            )

        def copyback(mq, n, ps):
            ot = outp.tile([P, NT], f32)
            nc.vector.tensor_tensor(
                ot[:], ps[:], b_rep[:, n * NT : (n + 1) * NT], mybir.AluOpType.add
            )
            chained_dma(
                (2 * mq + n) % 3,
                out_nd[mq * P : (mq + 1) * P, n * NT : (n + 1) * NT],
                ot[:],
            )

        # Phase A: rows 0-511, ko-outer across all 8 PSUM banks; step ko
        # fires as soon as DMA pair ko lands.
        pssA = {
            (m, n): psp.tile([P, NT], f32, tag="t", name=f"gA_{m}_{n}")
            for m in range(4)
            for n in range(2)
        }
        for ko in range(KO):
            for m in range(4):
                for n in range(2):
                    nc.tensor.matmul(
                        pssA[(m, n)][:],
                        lhsT=xt_sb[:, ko, m * P : (m + 1) * P],
                        rhs=w_sb[:, ko, n * NT : (n + 1) * NT],
                        start=(ko == 0),
                        stop=(ko == KO - 1),
                    )
        # copy banks in the order phase B will want them back
        for m in range(4):
            for n in range(2):
                copyback(m, n, pssA[(m, n)])

        # Phase B: rows 512-1023, K-contiguous per row-tile (all data
        # resident); output tiles complete early and stream out.
        for m in range(4, RT):
            pss = [psp.tile([P, NT], f32, tag="t", name=f"gB_{m}_{n}") for n in range(2)]
            for ko in range(KO):
                for n in range(2):
                    nc.tensor.matmul(
                        pss[n][:],
                        lhsT=xt_sb[:, ko, m * P : (m + 1) * P],
                        rhs=w_sb[:, ko, n * NT : (n + 1) * NT],
                        start=(ko == 0),
                        stop=(ko == KO - 1),
                    )
            for n in range(2):
                copyback(m, n, pss[n])

    nc.compile()
    _NC_CACHE["nc"] = nc
    return nc


def make_in_maps(inputs):
    import ml_dtypes

    bf16 = ml_dtypes.bfloat16

    wv = np.asarray(inputs["Wv"], dtype=np.float64)
    bv = np.asarray(inputs["bv"], dtype=np.float64)
    wo = np.asarray(inputs["Wo"], dtype=np.float64)
    bo = np.asarray(inputs["bo"], dtype=np.float64)
    wf = (2048.0 * (wv @ wo)).astype(np.float32).astype(bf16)
    bf = ((2048.0 * (bv @ wo)) + bo).astype(np.float32)

    xf = np.asarray(inputs["x"], dtype=np.float32).reshape(ROWS, D).astype(bf16)
    return [
        {
            "xt": np.ascontiguousarray(xf[c * R : (c + 1) * R].T),
            "w": wf,
            "b": bf,
        }
        for c in range(NCORES)
    ]


def kernel(**inputs) -> np.ndarray:
    from concourse.bass_utils import run_bass_kernel_spmd

    nc = build_nc()
    in_maps = make_in_maps(inputs)
    res = run_bass_kernel_spmd(nc, in_maps, list(range(NCORES)))
    out = np.concatenate(
        [res.results[c]["out"] for c in range(NCORES)], axis=0
    ).reshape(B, L, D)
    return np.ascontiguousarray(out.astype(np.float32, copy=False))


# revision 5
# speedup vs baseline: 1.0399x; 1.0399x over previous
"""Trainium2 Bass kernel for nn_CrossAttention_43061342110469.

Mathematical reduction: the reference's second einsum
    attn = einsum('bvhd,bhqk->bvhd', v, scores)
shares no contraction index with v, so it multiplies v elementwise by
S[b,h] = sum_{q,k} scores[b,h,q,k].  scores is a softmax over k, so every
row sums to 1 and S[b,h] == L == 2048 (verified: the fp32 reference
computes S == 2048.0 bit-exactly; end-to-end rel-err of the reduction is
~5e-7).  Therefore:

    out = (x @ Wv + bv) * 2048 @ Wo + bo
        = x @ (2048 * Wv @ Wo) + (2048 * bv @ Wo + bo)
        = x @ W' + b'

W' (1024x1024) and b' are folded on the host (float64 GEMM, ~ms), so the
device runs a single 8192x1024x1024 GEMM, row-sharded: 1024 rows per
core.  x and W' are cast to bf16 on the host (rel-err of the bf16 path vs
the fp32 reference is ~2.9e-3, tolerance 2e-2); x is pre-transposed
per-shard on the host (matmul wants the contraction dim on partitions for
both operands).

The device computes out^T[dout, row] (lhsT = W' tile, rhs = x^T slice) so
the bias is a per-partition scalar (4 KB load, no broadcast DMA; DVE
tensor_scalar and ACT activation-add can then both do PSUM copybacks in
parallel).  The host transposes out^T back and upcasts to f32.

Per-core device program:
  - DMA in: x^T [1024d, 1024r] bf16; W' split column-wise into wa
    (douts 0-511, needed first) and wb (douts 512-1023, needed ~15us
    later); chunks sized 128-512 KB in need-order round-robin over the 3
    issue queues with depth-2 completion chains.
  - ~2us of dummy matmuls from t0 so the PE HAM clock-gate opens while
    the first chunks land.
  - Phase A (douts 0-511): ko-outer accumulation over all 8 PSUM banks
    (4 dout-tiles x 2 512-row halves) so contraction step ko runs as
    soon as chunk pair ko lands.
  - Phase B (douts 512-1023): all data resident; K-contiguous per
    dout-tile so output tiles complete early and stream out.
  - Copybacks alternate DVE (row-half 0) and ACT (row-half 1) so freed
    PSUM banks are available ~0.7us after each phase-A stop; out tiles
    DMA out as 16 x 128KB bf16 transfers round-robin over the queues.
q/k/softmax are numerically dead and not computed.
"""

import sys

import numpy as np

_REPO = "/opt/trn_rl_repo"
if _REPO not in sys.path:
    sys.path.insert(0, _REPO)

B, L, D = 4, 2048, 1024
NCORES = 8
ROWS = B * L  # 8192
R = ROWS // NCORES  # 1024 rows per core
P = 128
NT = 512  # matmul free-dim tile (one PSUM bank of fp32)
KO = D // P  # 8 contraction tiles
JT = D // P  # 8 output-dim tiles
WHALF = D // 2  # 512: wa/wb column split

_NC_CACHE = {}


def build_nc():
    """Build + compile the per-core Bass program (cached)."""
    if "nc" in _NC_CACHE:
        return _NC_CACHE["nc"]

    from contextlib import ExitStack

    import concourse.tile as tile
    from concourse import bacc, mybir
    from concourse.tile_rust import add_dep_helper
    from concourse._compat import get_trn_type

    f32 = mybir.dt.float32
    bf16 = mybir.dt.bfloat16

    nc = bacc.Bacc(
        get_trn_type() or "TRN2",
        target_bir_lowering=False,
        debug=False,
        num_devices=NCORES,
    )

    xt_nd = nc.dram_tensor("xt", [D, R], bf16, kind="ExternalInput").ap()
    wa_nd = nc.dram_tensor("wa", [D, WHALF], bf16, kind="ExternalInput").ap()
    wb_nd = nc.dram_tensor("wb", [D, WHALF], bf16, kind="ExternalInput").ap()
    b_nd = nc.dram_tensor("b", [D], f32, kind="ExternalInput").ap()
    out_nd = nc.dram_tensor("out", [D, R], bf16, kind="ExternalOutput").ap()

    with tile.TileContext(nc) as tc, ExitStack() as ctx:
        const = ctx.enter_context(tc.tile_pool(name="const", bufs=1))
        big = ctx.enter_context(tc.tile_pool(name="big", bufs=1))
        psp = ctx.enter_context(tc.tile_pool(name="psp", bufs=8, space="PSUM"))
        outp = ctx.enter_context(tc.tile_pool(name="outp", bufs=4))

        xt_sb = big.tile([P, KO, R], bf16)  # x^T as [d_in, d_out, row]
        wa_sb = big.tile([P, KO, WHALF], bf16)  # W'[:, :512] as [p, ko, n]
        wb_sb = big.tile([P, KO, WHALF], bf16)  # W'[:, 512:] as [p, ko, n]
        b2 = const.tile([P, JT], f32)  # b2[p, j] = b'[j*128 + p]

        # --- DMA schedule, in need-order: bias, then (wa[ko], xt[ko])
        # pairs sized small at the head (phase A round ko fires on pair ko)
        # and larger at the tail, then wb (phase B, needed ~15us later).
        # Round-robin over the 3 issue queues, depth-2 completion chains
        # (without them every dma_start floods the 16-engine fabric at once
        # and early chunks land late).
        xt_r = xt_nd.rearrange("(ko p) r -> p ko r", p=P)
        wa_r = wa_nd.rearrange("(ko p) n -> p ko n", p=P)
        wb_r = wb_nd.rearrange("(ko p) n -> p ko n", p=P)
        jobs = [
            (b2[:], b_nd.rearrange("(o p) -> p o", p=P)),
            (wa_sb[:, 0], wa_r[:, 0]),
            (xt_sb[:, 0], xt_r[:, 0]),
            (wa_sb[:, 1], wa_r[:, 1]),
            (xt_sb[:, 1], xt_r[:, 1]),
            (wa_sb[:, 2:4], wa_r[:, 2:4]),
            (xt_sb[:, 2:4], xt_r[:, 2:4]),
            (wa_sb[:, 4:6], wa_r[:, 4:6]),
            (xt_sb[:, 4:6], xt_r[:, 4:6]),
            (wa_sb[:, 6:8], wa_r[:, 6:8]),
            (xt_sb[:, 6:8], xt_r[:, 6:8]),
            (wb_sb[:, 0:4], wb_r[:, 0:4]),
            (wb_sb[:, 4:8], wb_r[:, 4:8]),
        ]

        qs = [nc.sync, nc.scalar, nc.gpsimd]
        chains = [[], [], []]

        def chained_dma(qi, dst, srcap):
            inst = qs[qi].dma_start(dst, srcap)
            ch = chains[qi]
            if len(ch) == 1:
                add_dep_helper(inst.ins, ch[-1].ins, sync=True, reason="dma chain")
            elif len(ch) >= 2:
                add_dep_helper(inst.ins, ch[-2].ins, sync=True, reason="dma chain")
            ch.append(inst)
            return inst

        for i, (dst, srcap) in enumerate(jobs):
            chained_dma(i % 3, dst, srcap)

        # --- PE warmup: dummy matmuls from t0 so the HAM clock-gate opens
        # (K=8/8 @ 2.4GHz) around the time real work arrives.
        warm = const.tile([P, P], bf16)
        nc.vector.memset(warm[:], 1.0)
        wps = psp.tile([P, NT], f32, tag="t", name="wps")
        for _ in range(24):
            nc.tensor.matmul(
                wps[:, 0:P], lhsT=warm[:], rhs=warm[:], start=True, stop=True
            )

        outs = {}

        def copyback(j, n, ps):
            if n == 0:
                ot = outp.tile([P, R], bf16)
                outs[j] = ot
                nc.vector.tensor_scalar_add(
                    ot[:, 0:NT], ps[:], b2[:, j : j + 1]
                )
            else:
                ot = outs[j]
                nc.scalar.add(ot[:, NT:R], ps[:], b2[:, j : j + 1])
            chained_dma(
                (2 * j + n) % 3,
                out_nd[j * P : (j + 1) * P, n * NT : (n + 1) * NT],
                ot[:, n * NT : (n + 1) * NT],
            )

        # Phase A: douts 0-511, ko-outer across all 8 PSUM banks; step ko
        # fires as soon as DMA pair ko lands.
        pssA = {
            (j, n): psp.tile([P, NT], f32, tag="t", name=f"gA_{j}_{n}")
            for j in range(4)
            for n in range(2)
        }
        for ko in range(KO):
            for j in range(4):
                for n in range(2):
                    nc.tensor.matmul(
                        pssA[(j, n)][:],
                        lhsT=wa_sb[:, ko, j * P : (j + 1) * P],
                        rhs=xt_sb[:, ko, n * NT : (n + 1) * NT],
                        start=(ko == 0),
                        stop=(ko == KO - 1),
                    )
        # copy banks in the order phase B will want them back
        for j in range(4):
            for n in range(2):
                copyback(j, n, pssA[(j, n)])

        # Phase B: douts 512-1023, K-contiguous per dout-tile (all data
        # resident); output tiles complete early and stream out.
        for j in range(4, JT):
            pss = [psp.tile([P, NT], f32, tag="t", name=f"gB_{j}_{n}") for n in range(2)]
            for ko in range(KO):
                for n in range(2):
                    nc.tensor.matmul(
                        pss[n][:],
                        lhsT=wb_sb[:, ko, (j - 4) * P : (j - 3) * P],
                        rhs=xt_sb[:, ko, n * NT : (n + 1) * NT],
                        start=(ko == 0),
                        stop=(ko == KO - 1),
                    )
            for n in range(2):
                copyback(j, n, pss[n])

    nc.compile()
    _NC_CACHE["nc"] = nc
    return nc


def make_in_maps(inputs):
    import ml_dtypes

    bf16 = ml_dtypes.bfloat16

    wv = np.asarray(inputs["Wv"], dtype=np.float64)
    bv = np.asarray(inputs["bv"], dtype=np.float64)
    wo = np.asarray(inputs["Wo"], dtype=np.float64)
    bo = np.asarray(inputs["bo"], dtype=np.float64)
    wf = (2048.0 * (wv @ wo)).astype(np.float32).astype(bf16)
    bf = ((2048.0 * (bv @ wo)) + bo).astype(np.float32)

    xf = np.asarray(inputs["x"], dtype=np.float32).reshape(ROWS, D).astype(bf16)
    wa = np.ascontiguousarray(wf[:, :WHALF])
    wb = np.ascontiguousarray(wf[:, WHALF:])
    return [
        {
            "xt": np.ascontiguousarray(xf[c * R : (c + 1) * R].T),
            "wa": wa,
            "wb": wb,
            "b": bf,
        }
        for c in range(NCORES)
    ]


def kernel(**inputs) -> np.ndarray:
    from concourse.bass_utils import run_bass_kernel_spmd

    nc = build_nc()
    in_maps = make_in_maps(inputs)
    res = run_bass_kernel_spmd(nc, in_maps, list(range(NCORES)))
    out = np.concatenate(
        [np.asarray(res.results[c]["out"]).T.astype(np.float32) for c in range(NCORES)],
        axis=0,
    ).reshape(B, L, D)
    return np.ascontiguousarray(out)


# revision 7
# speedup vs baseline: 1.0448x; 1.0047x over previous
"""Trainium2 Bass kernel for nn_CrossAttention_43061342110469.

Mathematical reduction: the reference's second einsum
    attn = einsum('bvhd,bhqk->bvhd', v, scores)
shares no contraction index with v, so it multiplies v elementwise by
S[b,h] = sum_{q,k} scores[b,h,q,k].  scores is a softmax over k, so every
row sums to 1 and S[b,h] == L == 2048 (verified: the fp32 reference
computes S == 2048.0 bit-exactly; end-to-end rel-err of the reduction is
~5e-7).  Therefore:

    out = (x @ Wv + bv) * 2048 @ Wo + bo
        = x @ (2048 * Wv @ Wo) + (2048 * bv @ Wo + bo)
        = x @ W' + b'

W' (1024x1024) and b' are folded on the host (float64 GEMM, ~ms), so the
device runs a single 8192x1024x1024 GEMM, row-sharded: 1024 rows per
core.  x and W' are cast to bf16 on the host (rel-err of the bf16 path vs
the fp32 reference is ~2.9e-3, tolerance 2e-2); x is pre-transposed
per-shard on the host (matmul wants the contraction dim on partitions for
both operands).

The device computes out^T[dout, row] (lhsT = W' tile, rhs = x^T slice) so
the bias is a per-partition scalar (host-preshaped [128,8] tile, 4 KB;
DVE tensor_scalar and ACT activation-add then both do PSUM copybacks in
parallel).  The host transposes out^T back and upcasts to f32.

Per-core device program:
  - DMA in: x^T [1024d, 1024r] bf16; W' split column-wise into wa
    (douts 0-511, needed first) and wb (douts 512-1023, needed ~15us
    later).  Three explicit per-queue streams in need-order (small chunks
    at the head so round 0 starts early, larger later), depth-3
    completion chains so each queue's descriptor generation pipelines;
    all three queues streaming concurrently reach the ~430 GB/s fabric
    ceiling.
  - ~2.6us of dummy matmuls from t0 so the PE HAM clock-gate opens while
    the first chunks land.
  - Phase A (douts 0-511): ko-outer accumulation over all 8 PSUM banks
    (4 dout-tiles x 2 512-row halves) so contraction step ko runs as
    soon as chunk pair ko lands.
  - Phase B (douts 512-1023): all data resident; K-contiguous per
    dout-tile so output tiles complete early and stream out.  The final
    dout-tile is split into 4 x 256-row sub-groups so the last
    copyback+DMA tail is ~64KB instead of ~256KB.
  - Copybacks alternate DVE (even) and ACT (odd) so freed PSUM banks are
    available ~0.7us after each phase-A stop.
q/k/softmax are numerically dead and not computed.
"""

import sys

import numpy as np

_REPO = "/opt/trn_rl_repo"
if _REPO not in sys.path:
    sys.path.insert(0, _REPO)

B, L, D = 4, 2048, 1024
NCORES = 8
ROWS = B * L  # 8192
R = ROWS // NCORES  # 1024 rows per core
P = 128
NT = 512  # matmul free-dim tile (one PSUM bank of fp32)
KO = D // P  # 8 contraction tiles
JT = D // P  # 8 output-dim tiles
WHALF = D // 2  # 512: wa/wb column split

_NC_CACHE = {}


def build_nc():
    """Build + compile the per-core Bass program (cached)."""
    if "nc" in _NC_CACHE:
        return _NC_CACHE["nc"]

    from contextlib import ExitStack

    import concourse.tile as tile
    from concourse import bacc, mybir
    from concourse.tile_rust import add_dep_helper
    from concourse._compat import get_trn_type

    f32 = mybir.dt.float32
    bf16 = mybir.dt.bfloat16

    nc = bacc.Bacc(
        get_trn_type() or "TRN2",
        target_bir_lowering=False,
        debug=False,
        num_devices=NCORES,
    )

    xt_nd = nc.dram_tensor("xt", [D, R], bf16, kind="ExternalInput").ap()
    wa_nd = nc.dram_tensor("wa", [D, WHALF], bf16, kind="ExternalInput").ap()
    wb_nd = nc.dram_tensor("wb", [D, WHALF], bf16, kind="ExternalInput").ap()
    b2_nd = nc.dram_tensor("b2", [P, JT], f32, kind="ExternalInput").ap()
    out_nd = nc.dram_tensor("out", [D, R], bf16, kind="ExternalOutput").ap()

    with tile.TileContext(nc) as tc, ExitStack() as ctx:
        const = ctx.enter_context(tc.tile_pool(name="const", bufs=1))
        big = ctx.enter_context(tc.tile_pool(name="big", bufs=1))
        psp = ctx.enter_context(tc.tile_pool(name="psp", bufs=8, space="PSUM"))
        outp = ctx.enter_context(tc.tile_pool(name="outp", bufs=4))

        xt_sb = big.tile([P, KO, R], bf16)  # x^T as [d_in, d_out, row]
        wa_sb = big.tile([P, KO, WHALF], bf16)  # W'[:, :512] as [p, ko, n]
        wb_sb = big.tile([P, KO, WHALF], bf16)  # W'[:, 512:] as [p, ko, n]
        b2 = const.tile([P, JT], f32)  # b2[p, j] = b'[j*128 + p]

        xt_r = xt_nd.rearrange("(ko p) r -> p ko r", p=P)
        wa_r = wa_nd.rearrange("(ko p) n -> p ko n", p=P)
        wb_r = wb_nd.rearrange("(ko p) n -> p ko n", p=P)

        # --- DMA schedule: three explicit per-queue streams in need-order.
        # Phase-A round ko needs (wa[ko], xt[ko]); heads are 128KB so round
        # 0 can start ~1us earlier; wb and b2 are needed only ~15us in.
        qjobs = [
            # sync queue
            [
                (xt_sb[:, 0, 0:NT], xt_r[:, 0, 0:NT]),
                (xt_sb[:, 1], xt_r[:, 1]),
                (xt_sb[:, 4], xt_r[:, 4]),
                (xt_sb[:, 6], xt_r[:, 6]),
                (wb_sb[:, 0:4], wb_r[:, 0:4]),
            ],
            # scalar queue
            [
                (wa_sb[:, 0], wa_r[:, 0]),
                (wa_sb[:, 2:4], wa_r[:, 2:4]),
                (xt_sb[:, 3], xt_r[:, 3]),
                (xt_sb[:, 7], xt_r[:, 7]),
                (wb_sb[:, 4:8], wb_r[:, 4:8]),
            ],
            # gpsimd queue
            [
                (xt_sb[:, 0, NT:R], xt_r[:, 0, NT:R]),
                (wa_sb[:, 1], wa_r[:, 1]),
                (xt_sb[:, 2], xt_r[:, 2]),
                (wa_sb[:, 4:6], wa_r[:, 4:6]),
                (wa_sb[:, 6:8], wa_r[:, 6:8]),
                (xt_sb[:, 5], xt_r[:, 5]),
                (b2[:], b2_nd),
            ],
        ]

        qs = [nc.sync, nc.scalar, nc.gpsimd]
        chains = [[], [], []]

        def chained_dma(qi, dst, srcap):
            inst = qs[qi].dma_start(dst, srcap)
            ch = chains[qi]
            if len(ch) >= 3:
                add_dep_helper(inst.ins, ch[-3].ins, sync=True, reason="dma chain")
            ch.append(inst)
            return inst

        for qi, jobs in enumerate(qjobs):
            for dst, srcap in jobs:
                chained_dma(qi, dst, srcap)

        # --- PE warmup: dummy matmuls from t0 so the HAM clock-gate opens
        # (K=8/8 @ 2.4GHz) around the time real work arrives.
        warm = const.tile([P, P], bf16)
        nc.vector.memset(warm[:], 1.0)
        wps = psp.tile([P, NT], f32, tag="t", name="wps")
        for _ in range(24):
            nc.tensor.matmul(
                wps[:, 0:P], lhsT=warm[:], rhs=warm[:], start=True, stop=True
            )

        outs = {}

        def copyback(j, lo, hi, ps, engine):
            if j not in outs:
                outs[j] = outp.tile([P, R], bf16, name=f"ot{j}")
            ot = outs[j]
            if engine == 0:
                nc.vector.tensor_scalar_add(ot[:, lo:hi], ps[:, 0 : hi - lo], b2[:, j : j + 1])
            else:
                nc.scalar.add(ot[:, lo:hi], ps[:, 0 : hi - lo], b2[:, j : j + 1])
            chained_dma(
                (j + lo // NT) % 3,
                out_nd[j * P : (j + 1) * P, lo:hi],
                ot[:, lo:hi],
            )

        # Phase A: douts 0-511, ko-outer across all 8 PSUM banks; step ko
        # fires as soon as DMA pair ko lands.
        pssA = {
            (j, n): psp.tile([P, NT], f32, tag="t", name=f"gA_{j}_{n}")
            for j in range(4)
            for n in range(2)
        }
        for ko in range(KO):
            for j in range(4):
                for n in range(2):
                    nc.tensor.matmul(
                        pssA[(j, n)][:],
                        lhsT=wa_sb[:, ko, j * P : (j + 1) * P],
                        rhs=xt_sb[:, ko, n * NT : (n + 1) * NT],
                        start=(ko == 0),
                        stop=(ko == KO - 1),
                    )
        # copy banks in the order phase B will want them back
        for j in range(4):
            for n in range(2):
                copyback(j, n * NT, (n + 1) * NT, pssA[(j, n)], n % 2)

        # Phase B: douts 512-895, K-contiguous per dout-tile (all data
        # resident); output tiles complete early and stream out.
        for j in range(4, 7):
            pss = [psp.tile([P, NT], f32, tag="t", name=f"gB_{j}_{n}") for n in range(2)]
            for ko in range(KO):
                for n in range(2):
                    nc.tensor.matmul(
                        pss[n][:],
                        lhsT=wb_sb[:, ko, (j - 4) * P : (j - 3) * P],
                        rhs=xt_sb[:, ko, n * NT : (n + 1) * NT],
                        start=(ko == 0),
                        stop=(ko == KO - 1),
                    )
            for n in range(2):
                copyback(j, n * NT, (n + 1) * NT, pss[n], n % 2)

        # Final dout-tile: 4 x 256-row K-contiguous sub-groups so the last
        # copyback+DMA after the final matmul is only ~64KB.
        j = 7
        for q in range(4):
            ps = psp.tile([P, NT], f32, tag="t", name=f"gB7_{q}")
            for ko in range(KO):
                nc.tensor.matmul(
                    ps[:, 0:256],
                    lhsT=wb_sb[:, ko, 3 * P : 4 * P],
                    rhs=xt_sb[:, ko, q * 256 : (q + 1) * 256],
                    start=(ko == 0),
                    stop=(ko == KO - 1),
                )
            copyback(j, q * 256, (q + 1) * 256, ps, q % 2)

    nc.compile()
    _NC_CACHE["nc"] = nc
    return nc


def make_in_maps(inputs):
    import ml_dtypes

    bf16 = ml_dtypes.bfloat16

    wv = np.asarray(inputs["Wv"], dtype=np.float64)
    bv = np.asarray(inputs["bv"], dtype=np.float64)
    wo = np.asarray(inputs["Wo"], dtype=np.float64)
    bo = np.asarray(inputs["bo"], dtype=np.float64)
    wf = (2048.0 * (wv @ wo)).astype(np.float32).astype(bf16)
    bf = ((2048.0 * (bv @ wo)) + bo).astype(np.float32)

    xf = np.asarray(inputs["x"], dtype=np.float32).reshape(ROWS, D).astype(bf16)
    wa = np.ascontiguousarray(wf[:, :WHALF])
    wb = np.ascontiguousarray(wf[:, WHALF:])
    b2 = np.ascontiguousarray(bf.reshape(JT, P).T)  # b2[p, j] = b'[j*128+p]
    return [
        {
            "xt": np.ascontiguousarray(xf[c * R : (c + 1) * R].T),
            "wa": wa,
            "wb": wb,
            "b2": b2,
        }
        for c in range(NCORES)
    ]


def kernel(**inputs) -> np.ndarray:
    from concourse.bass_utils import run_bass_kernel_spmd

    nc = build_nc()
    in_maps = make_in_maps(inputs)
    res = run_bass_kernel_spmd(nc, in_maps, list(range(NCORES)))
    out = np.concatenate(
        [np.asarray(res.results[c]["out"]).T.astype(np.float32) for c in range(NCORES)],
        axis=0,
    ).reshape(B, L, D)
    return np.ascontiguousarray(out)


# revision 12
# speedup vs baseline: 1.0833x; 1.0368x over previous
"""Trainium2 Bass kernel for nn_CrossAttention_43061342110469.

Mathematical reduction: the reference's second einsum
    attn = einsum('bvhd,bhqk->bvhd', v, scores)
shares no contraction index with v, so it multiplies v elementwise by
S[b,h] = sum_{q,k} scores[b,h,q,k].  scores is a softmax over k, so every
row sums to 1 and S[b,h] == L == 2048 (verified: the fp32 reference
computes S == 2048.0 bit-exactly; end-to-end rel-err of the reduction is
~5e-7).  Therefore:

    out = (x @ Wv + bv) * 2048 @ Wo + bo
        = x @ (2048 * Wv @ Wo) + (2048 * bv @ Wo + bo)
        = x @ W' + b'

W' (1024x1024) and b' are folded on the host (float64 GEMM, ~ms), so the
device runs a single 8192x1024x1024 GEMM, row-sharded: 1024 rows per
core.  x and W' are cast to bf16 on the host (rel-err of the bf16 path vs
the fp32 reference is ~2.9e-3, tolerance 2e-2); x is pre-transposed
per-shard on the host (matmul wants the contraction dim on partitions for
both operands).

The device computes out^T[dout, row] (lhsT = W' tile, rhs = x^T slice) so
the bias is a per-partition scalar (host-preshaped [128,8] tile, 4 KB;
DVE tensor_scalar and ACT activation-add then both do PSUM copybacks in
parallel).  The host transposes out^T back and upcasts to f32.

Per-core device program:
  - DMA in: x^T [1024d, 1024r] bf16; W' split column-wise into wa
    (douts 0-511, needed first) and wb (douts 512-1023, needed ~15us
    later).  Three explicit per-queue streams in need-order (small chunks
    at the head so round 0 starts early, larger later), depth-3
    completion chains so each queue's descriptor generation pipelines;
    all three queues streaming concurrently reach the ~430 GB/s fabric
    ceiling.
  - ~2.6us of dummy matmuls from t0 so the PE HAM clock-gate opens while
    the first chunks land.
  - Phase A (douts 0-511): ko-outer accumulation over all 8 PSUM banks
    (4 dout-tiles x 2 512-row halves) so contraction step ko runs as
    soon as chunk pair ko lands.
  - Phase B (douts 512-1023): all data resident; K-contiguous per
    dout-tile so output tiles complete early and stream out.  The final
    dout-tile is split into 4 x 256-row sub-groups so the last
    copyback+DMA tail is ~64KB instead of ~256KB.
  - Copybacks alternate DVE (even) and ACT (odd) so freed PSUM banks are
    available ~0.7us after each phase-A stop.
q/k/softmax are numerically dead and not computed.
"""

import sys

import numpy as np

_REPO = "/opt/trn_rl_repo"
if _REPO not in sys.path:
    sys.path.insert(0, _REPO)

B, L, D = 4, 2048, 1024
NCORES = 8
ROWS = B * L  # 8192
R = ROWS // NCORES  # 1024 rows per core
P = 128
NT = 512  # matmul free-dim tile (one PSUM bank of fp32)
KO = D // P  # 8 contraction tiles
JT = D // P  # 8 output-dim tiles
WHALF = D // 2  # 512: wa/wb column split

_NC_CACHE = {}


def build_nc():
    """Build + compile the per-core Bass program (cached)."""
    if "nc" in _NC_CACHE:
        return _NC_CACHE["nc"]

    from contextlib import ExitStack

    import concourse.tile as tile
    from concourse import bacc, mybir
    from concourse.tile_rust import add_dep_helper
    from concourse._compat import get_trn_type

    f32 = mybir.dt.float32
    bf16 = mybir.dt.bfloat16

    nc = bacc.Bacc(
        get_trn_type() or "TRN2",
        target_bir_lowering=False,
        debug=False,
        num_devices=NCORES,
    )

    xt_nd = nc.dram_tensor("xt", [D, R], bf16, kind="ExternalInput").ap()
    wa_nd = nc.dram_tensor("wa", [D, WHALF], bf16, kind="ExternalInput").ap()
    wb_nd = nc.dram_tensor("wb", [D, WHALF], bf16, kind="ExternalInput").ap()
    b2_nd = nc.dram_tensor("b2", [P, JT], f32, kind="ExternalInput").ap()
    out_nd = nc.dram_tensor("out", [D, R], bf16, kind="ExternalOutput").ap()

    with tile.TileContext(nc) as tc, ExitStack() as ctx:
        const = ctx.enter_context(tc.tile_pool(name="const", bufs=1))
        big = ctx.enter_context(tc.tile_pool(name="big", bufs=1))
        psp = ctx.enter_context(tc.tile_pool(name="psp", bufs=8, space="PSUM"))
        outp = ctx.enter_context(tc.tile_pool(name="outp", bufs=4))

        xt_sb = big.tile([P, KO, R], bf16)  # x^T as [d_in, d_out, row]
        wa_sb = big.tile([P, KO, WHALF], bf16)  # W'[:, :512] as [p, ko, n]
        wb_sb = big.tile([P, KO, WHALF], bf16)  # W'[:, 512:] as [p, ko, n]
        b2 = const.tile([P, JT], f32)  # b2[p, j] = b'[j*128 + p]

        xt_r = xt_nd.rearrange("(ko p) r -> p ko r", p=P)
        wa_r = wa_nd.rearrange("(ko p) n -> p ko n", p=P)
        wb_r = wb_nd.rearrange("(ko p) n -> p ko n", p=P)

        # --- DMA schedule: three explicit per-queue streams in need-order.
        # Phase-A round ko needs (wa[ko], xt[ko]); heads are 128KB so round
        # 0 can start ~1us earlier; wb and b2 are needed only ~15us in.
        qjobs = [
            # sync queue
            [
                (xt_sb[:, 0, 0:NT], xt_r[:, 0, 0:NT]),
                (xt_sb[:, 1], xt_r[:, 1]),
                (wb_sb[:, 0:4], wb_r[:, 0:4]),
                (xt_sb[:, 4], xt_r[:, 4]),
                (xt_sb[:, 6], xt_r[:, 6]),
            ],
            # scalar queue
            [
                (wa_sb[:, 0], wa_r[:, 0]),
                (wa_sb[:, 2:4], wa_r[:, 2:4]),
                (xt_sb[:, 3], xt_r[:, 3]),
                (wb_sb[:, 4:8], wb_r[:, 4:8]),
                (xt_sb[:, 7], xt_r[:, 7]),
            ],
            # gpsimd queue
            [
                (xt_sb[:, 0, NT:R], xt_r[:, 0, NT:R]),
                (wa_sb[:, 1], wa_r[:, 1]),
                (xt_sb[:, 2], xt_r[:, 2]),
                (wa_sb[:, 4:6], wa_r[:, 4:6]),
                (xt_sb[:, 5], xt_r[:, 5]),
                (wa_sb[:, 6:8], wa_r[:, 6:8]),
                (b2[:], b2_nd),
            ],
        ]

        qs = [nc.sync, nc.scalar, nc.gpsimd]
        chains = [[], [], []]

        def chained_dma(qi, dst, srcap):
            inst = qs[qi].dma_start(dst, srcap)
            ch = chains[qi]
            if len(ch) >= 4:
                add_dep_helper(inst.ins, ch[-4].ins, sync=True, reason="dma chain")
            ch.append(inst)
            return inst

        for qi, jobs in enumerate(qjobs):
            for dst, srcap in jobs:
                chained_dma(qi, dst, srcap)

        # --- PE warmup: dummy matmuls from t0 so the HAM clock-gate opens
        # (K=8/8 @ 2.4GHz) around the time real work arrives.
        warm = const.tile([P, P], bf16)
        nc.vector.memset(warm[:], 1.0)
        wps = psp.tile([P, NT], f32, tag="t", name="wps")
        for _ in range(20):
            nc.tensor.matmul(
                wps[:, 0:P], lhsT=warm[:], rhs=warm[:], start=True, stop=True
            )

        outs = {}

        def copyback(j, lo, hi, ps, engine, qi=None):
            if j not in outs:
                outs[j] = outp.tile([P, R], bf16, name=f"ot{j}")
            ot = outs[j]
            if engine == 0:
                nc.vector.tensor_scalar_add(ot[:, lo:hi], ps[:, 0 : hi - lo], b2[:, j : j + 1])
            else:
                nc.scalar.add(ot[:, lo:hi], ps[:, 0 : hi - lo], b2[:, j : j + 1])
            chained_dma(
                (j + lo // NT) % 3 if qi is None else qi,
                out_nd[j * P : (j + 1) * P, lo:hi],
                ot[:, lo:hi],
            )

        # Phase A: douts 0-511, ko-outer across all 8 PSUM banks; step ko
        # fires as soon as DMA pair ko lands.
        pssA = {
            (j, n): psp.tile([P, NT], f32, tag="t", name=f"gA_{j}_{n}")
            for j in range(4)
            for n in range(2)
        }
        for ko in range(KO):
            for j in range(4):
                for n in range(2):
                    nc.tensor.matmul(
                        pssA[(j, n)][:],
                        lhsT=wa_sb[:, ko, j * P : (j + 1) * P],
                        rhs=xt_sb[:, ko, n * NT : (n + 1) * NT],
                        start=(ko == 0),
                        stop=(ko == KO - 1),
                    )
        # copy banks in the order phase B will want them back
        for j in range(4):
            for n in range(2):
                copyback(j, n * NT, (n + 1) * NT, pssA[(j, n)], n % 2)

        # Phase B: douts 512-895, K-contiguous per dout-tile (all data
        # resident); output tiles complete early and stream out.
        for j in range(4, 7):
            pss = [psp.tile([P, NT], f32, tag="t", name=f"gB_{j}_{n}") for n in range(2)]
            for ko in range(KO):
                for n in range(2):
                    nc.tensor.matmul(
                        pss[n][:],
                        lhsT=wb_sb[:, ko, (j - 4) * P : (j - 3) * P],
                        rhs=xt_sb[:, ko, n * NT : (n + 1) * NT],
                        start=(ko == 0),
                        stop=(ko == KO - 1),
                    )
            for n in range(2):
                copyback(j, n * NT, (n + 1) * NT, pss[n], n % 2)

        # Final dout-tile: 4 x 256-row K-contiguous sub-groups so the last
        # copyback+DMA after the final matmul is only ~64KB.
        j = 7
        for q in range(4):
            ps = psp.tile([P, NT], f32, tag="t", name=f"gB7_{q}")
            for ko in range(KO):
                nc.tensor.matmul(
                    ps[:, 0:256],
                    lhsT=wb_sb[:, ko, 3 * P : 4 * P],
                    rhs=xt_sb[:, ko, q * 256 : (q + 1) * 256],
                    start=(ko == 0),
                    stop=(ko == KO - 1),
                )
            copyback(j, q * 256, (q + 1) * 256, ps, q % 2, qi=q % 2)

    nc.compile()
    _NC_CACHE["nc"] = nc
    return nc


def make_in_maps(inputs):
    import ml_dtypes

    bf16 = ml_dtypes.bfloat16

    wv = np.asarray(inputs["Wv"], dtype=np.float64)
    bv = np.asarray(inputs["bv"], dtype=np.float64)
    wo = np.asarray(inputs["Wo"], dtype=np.float64)
    bo = np.asarray(inputs["bo"], dtype=np.float64)
    wf = (2048.0 * (wv @ wo)).astype(np.float32).astype(bf16)
    bf = ((2048.0 * (bv @ wo)) + bo).astype(np.float32)

    xf = np.asarray(inputs["x"], dtype=np.float32).reshape(ROWS, D).astype(bf16)
    wa = np.ascontiguousarray(wf[:, :WHALF])
    wb = np.ascontiguousarray(wf[:, WHALF:])
    b2 = np.ascontiguousarray(bf.reshape(JT, P).T)  # b2[p, j] = b'[j*128+p]
    return [
        {
            "xt": np.ascontiguousarray(xf[c * R : (c + 1) * R].T),
            "wa": wa,
            "wb": wb,
            "b2": b2,
        }
        for c in range(NCORES)
    ]


def kernel(**inputs) -> np.ndarray:
    from concourse.bass_utils import run_bass_kernel_spmd

    nc = build_nc()
    in_maps = make_in_maps(inputs)
    res = run_bass_kernel_spmd(nc, in_maps, list(range(NCORES)))
    out = np.concatenate(
        [np.asarray(res.results[c]["out"]).T.astype(np.float32) for c in range(NCORES)],
        axis=0,
    ).reshape(B, L, D)
    return np.ascontiguousarray(out)
